# revision 8
# baseline (speedup 1.0000x reference)
"""KV-cache scatter kernel for TRN2 (8 NeuronCores, batch-sharded),
implemented as a true in-place cache update via output-buffer donation.

Semantics (per batch element b, one NeuronCore each):
    idx = input_pos[b] - 1                       # (Q,) row indices
    k_out[b] = k_cache[b];  k_out[b, idx] = k_val[b]
    v_out[b] = v_cache[b];  v_out[b, idx] = v_val[b]

Key idea: the PJRT execute path donates caller-supplied buffers as the
NEFF's output buffers (the same mechanism concourse relies on to give
kernels pre-zeroed outputs). We donate the CACHE arrays as the initial
contents of k_out/v_out, so the 3/4 of the output that scatter doesn't
touch is already in place and the NEFF only moves the val rows:
8 MiB/core of payload instead of 32 MiB/core.

Two compiled programs, selected on the host per input:

FAST (idx == arange(0, Q) exactly, host-verified): out rows [0,Q) <- val
as pure contiguous DRAM->DRAM copies, spread across the two HWDGE queues
(sync, scalar) and four SWDGE rings (qPoolDynamic..qPoolDynamic3).

GENERIC (any indices): idx = input_pos - 1 computed on DVE, val rows
staged HBM->SBUF on the HWDGE queues, then gpsimd indirect-scatter DMA
(128 rows/instr) into the donated output.
"""

import glob
import os
import sys
import tempfile
import types
from contextlib import ExitStack

import numpy as np

import concourse.bacc as bacc
import concourse.bass as bass
import concourse.mybir as mybir
import concourse.tile as tile

# Hardcoded problem shape (nn_KVCache): B batches over 8 cores.
B, L, H, D, Q = 8, 4096, 16, 64, 1024
HD = H * D          # 1024 f32 per cache row (4 KiB)
P = 128             # SBUF partitions
NT = Q // P         # 8 val tiles of 128 rows
N_CORES = 8

_cache = {}


def _new_nc(num_swdge_queues=1, skip_init_barrier=False):
    """skip_init_barrier elides the all-engine rendezvous Bass.__init__
    emits after its per-engine preambles. Safe only for programs whose
    engines share no state (the raw fast path: each engine runs its own
    DMA + own semaphore); buys ~1 us of earlier DMA start."""
    if not skip_init_barrier:
        return bacc.Bacc(
            "TRN2",
            target_bir_lowering=False,
            debug=False,
            num_devices=N_CORES,
            num_swdge_queues=num_swdge_queues,
        )
    orig = bass.Bass.all_engine_barrier
    bass.Bass.all_engine_barrier = lambda self, *a, **k: None
    try:
        return bacc.Bacc(
            "TRN2",
            target_bir_lowering=False,
            debug=False,
            num_devices=N_CORES,
            num_swdge_queues=num_swdge_queues,
        )
    finally:
        bass.Bass.all_engine_barrier = orig


# Fast-path DMA schedule: one maximal DMA instruction per queue (per-queue
# rate rises with instruction size), raw Bass (no TileContext; the Bacc
# prologue alone costs ~7.5 us before the first packet — TileContext's
# barriers add nothing on top, but its epilogue does). sync HWDGE carries
# k[0:AK), scalar HWDGE v[0:AV), the single usable SWDGE ring (plain DMA
# copies are pinned to ring 0 by walrus alloc_queues) takes the remainders.
# HWDGE gets more than SWDGE (descgen ramps ~4 us earlier), but k and v
# shares are kept EQUAL: which HWDGE queue's descgen goes first is random
# run to run, so an asymmetric k/v split mismatches half the time. The
# SWDGE ring share (2*(Q-A) rows = 1.75 MiB at A=800) is sized so the
# ring tail matches the HWDGE pair's shared ~300 GB/s descgen ceiling
# even in the slow contention mode (runs are bimodal, ~35.5 or ~41 us).
FAST_AK = 800
FAST_AV = 800


def build_fast():
    nc = _new_nc(skip_init_barrier=True)
    kv = nc.dram_tensor("k_val", [Q, HD], mybir.dt.float32, kind="ExternalInput")
    vv = nc.dram_tensor("v_val", [Q, HD], mybir.dt.float32, kind="ExternalInput")
    ko = nc.dram_tensor("k_out", [L, HD], mybir.dt.float32, kind="ExternalOutput")
    vo = nc.dram_tensor("v_out", [L, HD], mybir.dt.float32, kind="ExternalOutput")
    ak, av = FAST_AK, FAST_AV

    sem_g = nc.alloc_semaphore("dma_g")
    sem_s = nc.alloc_semaphore("dma_s")
    sem_a = nc.alloc_semaphore("dma_a")

    # each engine: clear own sem, issue copies, wait for DMA completion
    # (+16/instr); no cross-engine sync needed
    nc.gpsimd.sem_clear(sem_g)
    nc.gpsimd.dma_start(out=ko[ak:Q, :], in_=kv[ak:Q, :]).then_inc(sem_g, 16)
    nc.gpsimd.dma_start(out=vo[av:Q, :], in_=vv[av:Q, :]).then_inc(sem_g, 16)
    nc.gpsimd.wait_ge(sem_g, 32)

    nc.sync.sem_clear(sem_s)
    nc.sync.dma_start(out=ko[0:ak, :], in_=kv[0:ak, :]).then_inc(sem_s, 16)
    nc.sync.wait_ge(sem_s, 16)

    nc.scalar.sem_clear(sem_a)
    nc.scalar.dma_start(out=vo[0:av, :], in_=vv[0:av, :]).then_inc(sem_a, 16)
    nc.scalar.wait_ge(sem_a, 16)

    nc.compile()
    return nc


def build_generic():
    nc = _new_nc(num_swdge_queues=4)
    kv = nc.dram_tensor("k_val", [Q, HD], mybir.dt.float32, kind="ExternalInput")
    vv = nc.dram_tensor("v_val", [Q, HD], mybir.dt.float32, kind="ExternalInput")
    pos = nc.dram_tensor("pos", [Q, 1], mybir.dt.int32, kind="ExternalInput")
    ko = nc.dram_tensor("k_out", [L, HD], mybir.dt.float32, kind="ExternalOutput")
    vo = nc.dram_tensor("v_out", [L, HD], mybir.dt.float32, kind="ExternalOutput")

    with ExitStack() as ctx:
        tc = ctx.enter_context(tile.TileContext(nc))
        sp = ctx.enter_context(tc.tile_pool(name="sbuf", bufs=1))

        pos_sb = sp.tile([P, NT], dtype=mybir.dt.int32)
        idx_sb = sp.tile([P, NT], dtype=mybir.dt.int32)
        kval_sb = sp.tile([P, NT * HD], dtype=mybir.dt.float32)
        vval_sb = sp.tile([P, NT * HD], dtype=mybir.dt.float32)

        # pos_sb[p, j] = pos[j*P + p]; idx = pos - 1
        nc.sync.dma_start(out=pos_sb[:], in_=bass.AP(pos, 0, [[1, P], [P, NT]]))
        nc.vector.tensor_scalar_sub(idx_sb[:], pos_sb[:], 1)

        # val_sb[p, j*HD + c] = val[j*P + p, c]
        nc.sync.dma_start(
            out=kval_sb[:], in_=bass.AP(kv, 0, [[HD, P], [P * HD, NT], [1, HD]])
        )
        nc.scalar.dma_start(
            out=vval_sb[:], in_=bass.AP(vv, 0, [[HD, P], [P * HD, NT], [1, HD]])
        )

        # scatter: out[idx[p], :] = val_sb[p, tile j], round-robin SWDGE rings
        for n, (dst, val_sb) in enumerate([(ko, kval_sb), (vo, vval_sb)]):
            for j in range(NT):
                inst = nc.gpsimd.indirect_dma_start(
                    out=dst[:, :],
                    out_offset=bass.IndirectOffsetOnAxis(
                        ap=idx_sb[:, j : j + 1], axis=0
                    ),
                    in_=val_sb[:, j * HD : (j + 1) * HD],
                    in_offset=None,
                )
                ring = (n * NT + j) % 4
                if ring:
                    inst.ins.queue = f"qPoolDynamic{ring}"

    nc.compile()
    return nc


def _get_nc(which):
    if which not in _cache:
        _cache[which] = build_fast() if which == "fast" else build_generic()
    return _cache[which]


def _is_fast(input_pos):
    expect = np.broadcast_to(
        np.arange(1, Q + 1, dtype=np.int32), np.asarray(input_pos).shape
    )
    return np.array_equal(np.asarray(input_pos), expect)


def make_in_maps(k_cache, v_cache, k_val, v_val, input_pos, with_pos=False):
    """Global (concatenated over cores) input + donated-init arrays."""
    ins = {
        "k_val": np.ascontiguousarray(np.asarray(k_val)).reshape(B * Q, HD),
        "v_val": np.ascontiguousarray(np.asarray(v_val)).reshape(B * Q, HD),
    }
    if with_pos:
        ins["pos"] = np.ascontiguousarray(
            np.asarray(input_pos).astype(np.int32, copy=False)
        ).reshape(B * Q, 1)
    inits = {
        "k_out": np.ascontiguousarray(np.asarray(k_cache)).reshape(B * L, HD),
        "v_out": np.ascontiguousarray(np.asarray(v_cache)).reshape(B * L, HD),
    }
    return ins, inits


def _run_pjrt(nc, global_ins, global_inits):
    """run_bass_via_pjrt, but ExternalOutput buffers are donated from
    caller-provided per-output init arrays (global, concat over cores)
    instead of zeros. Untouched output regions keep the init contents."""
    import jax
    from concourse.bass2jax import (
        _bass_exec_p,
        install_neuronx_cc_hook,
        partition_id_tensor,
    )
    from jax.experimental.shard_map import shard_map
    from jax.sharding import Mesh, PartitionSpec

    install_neuronx_cc_hook()
    assert nc.dbg_addr is None, "kernel must be built with debug=False"
    partition_name = nc.partition_id_tensor.name if nc.partition_id_tensor else None

    in_names, out_names, out_avals = [], [], []
    for alloc in nc.m.functions[0].allocations:
        if not isinstance(alloc, mybir.MemoryLocationSet):
            continue
        name = alloc.memorylocations[0].name
        if alloc.kind == "ExternalInput":
            if name != partition_name:
                in_names.append(name)
        elif alloc.kind == "ExternalOutput":
            out_names.append(name)
            out_avals.append(
                jax.core.ShapedArray(
                    tuple(alloc.tensor_shape), mybir.dt.np(alloc.dtype)
                )
            )
    n_params = len(in_names)
    n_outs = len(out_names)
    bind_in_names = list(in_names) + list(out_names)
    if partition_name is not None:
        bind_in_names.append(partition_name)
    donate = tuple(range(n_params, n_params + n_outs))

    def _body(*args):
        operands = list(args)
        if partition_name is not None:
            operands.append(partition_id_tensor())
        outs = _bass_exec_p.bind(
            *operands,
            out_avals=tuple(out_avals),
            in_names=tuple(bind_in_names),
            out_names=tuple(out_names),
            lowering_input_output_aliases=(),
            sim_require_finite=True,
            sim_require_nnan=True,
            nc=nc,
        )
        return tuple(outs)

    devices = jax.devices()[:N_CORES]
    assert len(devices) == N_CORES, f"need {N_CORES} devices, got {len(devices)}"
    mesh = Mesh(np.asarray(devices), ("core",))
    in_specs = (PartitionSpec("core"),) * (n_params + n_outs)
    out_specs = (PartitionSpec("core"),) * n_outs
    sharded = jax.jit(
        shard_map(
            _body, mesh=mesh, in_specs=in_specs, out_specs=out_specs, check_rep=False
        ),
        donate_argnums=donate,
        keep_unused=True,
    )
    args = [global_ins[nm] for nm in in_names] + [global_inits[nm] for nm in out_names]
    out_arrs = sharded(*args)
    return {nm: np.asarray(out_arrs[i]) for i, nm in enumerate(out_names)}


def _install_hook_shim():
    """Register the NTFF profile hook concourse expects under axon; the
    image's antenv package lacks the axon_hooks module the boot would
    normally populate, so recreate it from trn_agent_boot's factory."""
    try:
        from antenv.axon_hooks import get_axon_ntff_profile_hook

        return get_axon_ntff_profile_hook()
    except ImportError:
        pass
    import antenv
    from trn_agent_boot.trn_boot import _ntff_profile_via_ctypes

    mod = types.ModuleType("antenv.axon_hooks")
    _store = {}
    mod.set_axon_ntff_profile_hook = lambda h: _store.__setitem__("hook", h)
    mod.get_axon_ntff_profile_hook = lambda: _store.get("hook")
    sys.modules["antenv.axon_hooks"] = mod
    antenv.axon_hooks = mod
    mod.set_axon_ntff_profile_hook(
        _ntff_profile_via_ctypes("/opt/axon/libaxon_pjrt.so")
    )
    return mod.get_axon_ntff_profile_hook()


class RunResult:
    def __init__(self, outs, exec_time_ns=None, instructions_and_trace=None,
                 profile_json=None):
        self.outs = outs
        self.exec_time_ns = exec_time_ns
        self.instructions_and_trace = instructions_and_trace
        self.profile_json = profile_json


def run(in_maps, which="fast", trace=False, trace_cores=None):
    """in_maps: (global_ins, global_inits) from make_in_maps."""
    global_ins, global_inits = in_maps
    nc = _get_nc(which)
    if not trace:
        return RunResult(_run_pjrt(nc, global_ins, global_inits))

    hook = _install_hook_shim()
    if hook is None:
        return RunResult(_run_pjrt(nc, global_ins, global_inits))
    neff_dir = tempfile.mkdtemp()
    with hook(neff_dir, list(trace_cores) if trace_cores else [0]):
        outs = _run_pjrt(nc, global_ins, global_inits)
    if not glob.glob(os.path.join(neff_dir, "*_body*.ntff")):
        return RunResult(outs)

    import gauge.profiler
    from concourse._compat import FishPath
    from concourse.bass_utils import _process_ntff_profile

    profile = gauge.profiler.Profile(
        profile_path=FishPath(neff_dir),
        kernel_dev_mode=True,
        profile_on_exit=False,
        bass_kernel=nc.m,
        offline_processing=True,
        fname="*_body*",
        metadata={},
    )
    r = _process_ntff_profile(
        profile, neff_dir, nc, list(range(N_CORES)), trace_cores, False, {},
        trace_events=False,
    )
    return RunResult(
        outs,
        exec_time_ns=r.exec_time_ns,
        instructions_and_trace=r.insts_and_trace_path,
        profile_json=r.profile_json,
    )


def kernel(k_cache, v_cache, k_val, v_val, input_pos):
    fast = _is_fast(input_pos)
    which = "fast" if fast else "generic"
    in_maps = make_in_maps(
        k_cache, v_cache, k_val, v_val, input_pos, with_pos=not fast
    )
    res = run(in_maps, which=which)
    k_out = res.outs["k_out"].reshape(B, L, H, D)
    v_out = res.outs["v_out"].reshape(B, L, H, D)
    return k_out.astype(np.float32, copy=False), v_out.astype(np.float32, copy=False)
